# revision 1
# baseline (speedup 1.0000x reference)
"""CrossAttentionLayer kernel for 8x Trainium2 NeuronCores.

Problem (hardcoded): B=2, S=4096, HIDDEN=4096, HEADS=32, HEAD_DIM=128,
SLOTS=128, LN eps 1e-5.  out = x + (softmax(LN(x)@Wq.T split-heads @ K.T
/ sqrt(128), masked) @ V merge-heads) @ Wout.T

Strategy: data-parallel over the 8192 (B*S) rows — 1024 rows per core,
core c takes batch c//4.  Everything on-device except layout prep:
  * transposed dataflow: x.T [k, s] tiles; LN stats (mean/var over k =
    partition axis) via ones-matmul partition-broadcast sums; xn = bf16
  * Q-proj: QT[n,s] = (Wq*gamma).T-stationary @ xn.T, bias (beta@Wq.T)
    added on psum->sbuf copy; fused per-head with attention
  * attention per head in [t,s]/[d,s] layout: scoresT = K_h.T-st @ QT_h;
    exp on ACT (mask as per-partition bias, 1/sqrt(128) as scale);
    denominator via ones-matmul; attnT = V_h-st @ expT, normalized by
    reciprocal on the psum->sbuf copy (bf16)
  * out-proj: outT[n,s] = Wout.T-tiles-stationary @ attnT accumulated
    over heads; residual added from x.T f32; output outT per core,
    transposed/concatenated on host.
All matmuls bf16 x bf16 -> fp32 psum, N=512.
"""
import numpy as np
import ml_dtypes
import concourse.bass as bass
import concourse.mybir as mybir
import concourse.tile as tile
from concourse.vector_clock import ScopedClock

F32 = mybir.dt.float32
BF16 = mybir.dt.bfloat16
AF = mybir.ActivationFunctionType

B, S, HID, HEADS, DH, SLOTS = 2, 4096, 4096, 32, 128, 128
NC_ = 8
SC = B * S // NC_          # rows per core = 1024
KT = HID // 128            # 32 k-tiles
NT = HID // 128            # 32 n-tiles (= heads for Q)
NSL = SC // 512            # 2 moving slices of 512
EPS = 1e-5
SCALE = DH ** -0.5

_ws_counter = [0]


def _split_waits(nc, maxw=1):
    """This walrus build rejects >1 sync-wait per instruction: hoist
    extras into same-engine no-ops placed just before the instruction."""
    n = 0
    for f in nc.m.functions:
        for blk in f.blocks:
            insts = list(blk.instructions)
            out, dirty = [], False
            for inst in insts:
                si = inst.sync_info
                waits = list(si.on_wait) if (si is not None and si.on_wait) else []
                if len(waits) > maxw:
                    ups = list(si.on_update or [])
                    for i in range(maxw, len(waits), maxw):
                        _ws_counter[0] += 1
                        nop = mybir.InstNoOp(
                            name=f"I-ws{_ws_counter[0]}", ins=[], outs=[]
                        )
                        nop.engine = inst.engine
                        nop.sync_info = mybir.SyncInfo(
                            on_wait=waits[i : i + maxw], on_update=[]
                        )
                        out.append(nop)
                        n += 1
                    inst.sync_info = mybir.SyncInfo(
                        on_wait=waits[:maxw], on_update=ups
                    )
                    dirty = True
                out.append(inst)
            if dirty:
                blk.instructions = out
    return n


def _patch_tile_drain():
    import concourse.tile as tile_mod

    def _patched(self, tick_clock, wait_clock):
        nc = self.nc
        drain_inst = nc.sync.drain()
        wait_clock.add_sem_waits(
            drain_inst.ins, ScopedClock({None: tick_clock.global_clock})
        )
        inst = drain_inst.ins
        si = inst.sync_info
        waits = list(si.on_wait or []) if si is not None else []
        if len(waits) > 1:
            ups = list(si.on_update or []) if si is not None else []
            inst.sync_info = mybir.SyncInfo(on_wait=waits[:1], on_update=ups)
            for i in range(1, len(waits)):
                nop = nc.sync.nop()
                nop.ins.sync_info = mybir.SyncInfo(
                    on_wait=waits[i : i + 1], on_update=[]
                )
        nc.all_engine_barrier()
        assert self.sems is not None
        popped = nc._tile_sem_poison_stack.pop()
        assert popped is self._sem_poison
        nc.clear_and_free_semaphores(list(self.sems.allocated().values()))
        nc.all_engine_barrier()

    tile_mod.TileContext._drain_and_barrier = _patched


def build_nc():
    _patch_tile_drain()
    nc = bass.Bass()

    xtb_in = nc.dram_tensor("xtb", [HID, SC], BF16, kind="ExternalInput")
    xtf_in = nc.dram_tensor("xtf", [HID, SC], F32, kind="ExternalInput")
    wqt_in = nc.dram_tensor("wqt", [HID, HID], BF16, kind="ExternalInput")
    bq_in = nc.dram_tensor("bq", [128, NT], F32, kind="ExternalInput")
    wot_in = nc.dram_tensor("wot", [HID, HID], BF16, kind="ExternalInput")
    kt_in = nc.dram_tensor("ktt", [HEADS, DH, SLOTS], BF16, kind="ExternalInput")
    v_in = nc.dram_tensor("vv", [HEADS, SLOTS, DH], BF16, kind="ExternalInput")
    mb_in = nc.dram_tensor("mb", [SLOTS, 1], F32, kind="ExternalInput")
    out_t = nc.dram_tensor("outt", [HID, SC], F32, kind="ExternalOutput")

    with tile.TileContext(nc) as tc:
        with tc.tile_pool(name="persist", bufs=1) as P:
            ones = P.tile([128, 128], BF16, tag="ones")
            nc.vector.memset(ones[:], 1.0)
            eps_t = P.tile([128, 1], F32, tag="eps")
            nc.vector.memset(eps_t[:], EPS)
            kt_all = P.tile([128, HEADS, SLOTS], BF16, tag="kt")
            nc.sync.dma_start(
                kt_all[:], kt_in[:].rearrange("h d t -> d h t")
            )
            v_all = P.tile([128, HEADS, DH], BF16, tag="v")
            nc.sync.dma_start(v_all[:], v_in[:].rearrange("h t d -> t h d"))
            mb = P.tile([128, 1], F32, tag="mb")
            nc.sync.dma_start(mb[:], mb_in[:])
            bq = P.tile([128, NT], F32, tag="bq")
            nc.sync.dma_start(bq[:], bq_in[:])

            rstd_b = P.tile([128, SC], F32, tag="rstd")
            mrs_b = P.tile([128, SC], F32, tag="mrs")
            xn = [P.tile([128, SC], BF16, tag=f"xn{k}", name=f"xn{k}") for k in range(KT)]
            att = [P.tile([128, SC], BF16, tag=f"att{h}", name=f"att{h}") for h in range(HEADS)]

            # ---------- phase 1: LN stats ----------
            with (
                tc.tile_pool(name="xs", bufs=6) as XS,
                tc.tile_pool(name="sqp", bufs=4) as SQ,
                tc.tile_pool(name="stps", bufs=1, space="PSUM") as STP,
                tc.tile_pool(name="stsb", bufs=2) as STS,
            ):
                sum_ps = [STP.tile([128, 512], F32, tag=f"sum{sl}", name=f"sum{sl}") for sl in range(NSL)]
                ssq_ps = [STP.tile([128, 512], F32, tag=f"ssq{sl}", name=f"ssq{sl}") for sl in range(NSL)]
                for k in range(KT):
                    xt = XS.tile([128, SC], BF16, tag="xt")
                    nc.sync.dma_start(xt[:], xtb_in[k * 128 : (k + 1) * 128, :])
                    sq = SQ.tile([128, SC], BF16, tag="sq")
                    nc.scalar.square(sq[:], xt[:])
                    for sl in range(NSL):
                        cs = slice(sl * 512, (sl + 1) * 512)
                        nc.tensor.matmul(
                            sum_ps[sl][:], ones[:], xt[:, cs],
                            start=(k == 0), stop=(k == KT - 1),
                        )
                        nc.tensor.matmul(
                            ssq_ps[sl][:], ones[:], sq[:, cs],
                            start=(k == 0), stop=(k == KT - 1),
                        )
                for sl in range(NSL):
                    cs = slice(sl * 512, (sl + 1) * 512)
                    mean = STS.tile([128, 512], F32, tag="mean")
                    nc.vector.tensor_scalar_mul(mean[:], sum_ps[sl][:], 1.0 / HID)
                    esq = STS.tile([128, 512], F32, tag="esq")
                    nc.vector.tensor_scalar_mul(esq[:], ssq_ps[sl][:], 1.0 / HID)
                    msq = STS.tile([128, 512], F32, tag="msq")
                    nc.vector.tensor_mul(msq[:], mean[:], mean[:])
                    var = STS.tile([128, 512], F32, tag="var")
                    nc.vector.tensor_sub(var[:], esq[:], msq[:])
                    std = STS.tile([128, 512], F32, tag="std")
                    nc.scalar.activation(std[:], var[:], AF.Sqrt, bias=eps_t[:])
                    nc.vector.reciprocal(rstd_b[:, cs], std[:])
                    nc.vector.tensor_mul(mrs_b[:, cs], mean[:], rstd_b[:, cs])

            # ---------- phase 2: xn ----------
            with (
                tc.tile_pool(name="xs2", bufs=6) as XS2,
                tc.tile_pool(name="tmpp", bufs=4) as TMP,
            ):
                for k in range(KT):
                    xt = XS2.tile([128, SC], BF16, tag="xt2")
                    nc.sync.dma_start(xt[:], xtb_in[k * 128 : (k + 1) * 128, :])
                    tmp = TMP.tile([128, SC], F32, tag="tmp")
                    nc.vector.tensor_mul(tmp[:], xt[:], rstd_b[:])
                    nc.vector.tensor_sub(xn[k][:], tmp[:], mrs_b[:])

            # ---------- phase 3: per-head Q-proj + attention ----------
            with (
                tc.tile_pool(name="wq", bufs=2) as WQ,
                tc.tile_pool(name="qps", bufs=1, space="PSUM") as QPS,
                tc.tile_pool(name="qsb", bufs=2) as QSB,
                tc.tile_pool(name="aps", bufs=1, space="PSUM") as APS,
                tc.tile_pool(name="expp", bufs=2) as EXP,
                tc.tile_pool(name="rcp", bufs=2) as RCP,
            ):
                for h in range(HEADS):
                    wq = WQ.tile([128, KT, 128], BF16, tag="wq")
                    nc.sync.dma_start(
                        wq[:],
                        wqt_in[:, h * 128 : (h + 1) * 128].rearrange(
                            "(kt p) n -> p kt n", p=128
                        ),
                    )
                    qt_ps = [QPS.tile([128, 512], F32, tag=f"qt{sl}", name=f"qtp{sl}") for sl in range(NSL)]
                    for k in range(KT):
                        for sl in range(NSL):
                            cs = slice(sl * 512, (sl + 1) * 512)
                            nc.tensor.matmul(
                                qt_ps[sl][:], wq[:, k, :], xn[k][:, cs],
                                start=(k == 0), stop=(k == KT - 1),
                            )
                    qt = QSB.tile([128, SC], BF16, tag="qt")
                    for sl in range(NSL):
                        cs = slice(sl * 512, (sl + 1) * 512)
                        nc.vector.tensor_scalar_add(
                            qt[:, cs], qt_ps[sl][:], bq[:, h : h + 1]
                        )
                    expt = EXP.tile([128, SC], BF16, tag="expt")
                    for sl in range(NSL):
                        cs = slice(sl * 512, (sl + 1) * 512)
                        sc_ps = APS.tile([128, 512], F32, tag=f"sc{sl}")
                        nc.tensor.matmul(
                            sc_ps[:], kt_all[:, h, :], qt[:, cs],
                            start=True, stop=True,
                        )
                        nc.scalar.activation(
                            expt[:, cs], sc_ps[:], AF.Exp,
                            bias=mb[:], scale=SCALE,
                        )
                    for sl in range(NSL):
                        cs = slice(sl * 512, (sl + 1) * 512)
                        den_ps = APS.tile([128, 512], F32, tag=f"den{sl}")
                        nc.tensor.matmul(
                            den_ps[:], ones[:], expt[:, cs], start=True, stop=True
                        )
                        rcp = RCP.tile([128, 512], F32, tag="rcp")
                        nc.vector.reciprocal(rcp[:], den_ps[:])
                        at_ps = APS.tile([128, 512], F32, tag=f"at{sl}")
                        nc.tensor.matmul(
                            at_ps[:], v_all[:, h, :], expt[:, cs],
                            start=True, stop=True,
                        )
                        nc.vector.tensor_mul(att[h][:, cs], at_ps[:], rcp[:])

            # ---------- phase 4: out-proj + residual ----------
            with (
                tc.tile_pool(name="wo", bufs=2) as WO,
                tc.tile_pool(name="ops", bufs=2, space="PSUM") as OPS,
                tc.tile_pool(name="xrs", bufs=2) as XRS,
                tc.tile_pool(name="osb", bufs=3) as OSB,
            ):
                for nt in range(NT):
                    wo = WO.tile([128, KT, 128], BF16, tag="wo")
                    nc.sync.dma_start(
                        wo[:],
                        wot_in[:, nt * 128 : (nt + 1) * 128].rearrange(
                            "(ht p) n -> p ht n", p=128
                        ),
                    )
                    xr = XRS.tile([128, SC], F32, tag="xr")
                    nc.sync.dma_start(xr[:], xtf_in[nt * 128 : (nt + 1) * 128, :])
                    o_ps = [OPS.tile([128, 512], F32, tag=f"o{sl}", name=f"op{sl}") for sl in range(NSL)]
                    for h in range(HEADS):
                        for sl in range(NSL):
                            cs = slice(sl * 512, (sl + 1) * 512)
                            nc.tensor.matmul(
                                o_ps[sl][:], wo[:, h, :], att[h][:, cs],
                                start=(h == 0), stop=(h == HEADS - 1),
                            )
                    for sl in range(NSL):
                        cs = slice(sl * 512, (sl + 1) * 512)
                        osb = OSB.tile([128, 512], F32, tag="osb")
                        nc.vector.tensor_add(osb[:], o_ps[sl][:], xr[:, cs])
                        nc.sync.dma_start(
                            out_t[nt * 128 : (nt + 1) * 128, cs], osb[:]
                        )

    _split_waits(nc)
    return nc


_NC_CACHE = None
_LAST_IN_MAPS = None


def kernel(
    hidden_states, memory_keys, memory_values, attention_mask, Wq, Wout,
    ln_gamma, ln_beta,
):
    global _NC_CACHE
    if _NC_CACHE is None:
        _NC_CACHE = build_nc()
    nc = _NC_CACHE

    f32 = np.float32
    bf16 = ml_dtypes.bfloat16
    x = np.asarray(hidden_states, dtype=f32).reshape(B * S, HID)
    gamma = np.asarray(ln_gamma, dtype=f32)
    beta = np.asarray(ln_beta, dtype=f32)
    Wq = np.asarray(Wq, dtype=f32)
    Wout = np.asarray(Wout, dtype=f32)

    wqt = np.ascontiguousarray((Wq * gamma[None, :]).T).astype(bf16)
    bq = np.ascontiguousarray((Wq @ beta).reshape(NT, 128).T).astype(f32)
    wot = np.ascontiguousarray(Wout.T).astype(bf16)

    kts, vs, mbs = [], [], []
    for b in range(B):
        kb = np.asarray(memory_keys[b], dtype=f32).reshape(SLOTS, HEADS, DH)
        vb = np.asarray(memory_values[b], dtype=f32).reshape(SLOTS, HEADS, DH)
        kts.append(np.ascontiguousarray(kb.transpose(1, 2, 0)).astype(bf16))
        vs.append(np.ascontiguousarray(vb.transpose(1, 0, 2)).astype(bf16))
        m = np.asarray(attention_mask[b]).astype(bool)
        mbs.append(np.where(m, 0.0, -1e30).astype(f32).reshape(SLOTS, 1))

    in_maps = []
    for c in range(NC_):
        rows = slice(c * SC, (c + 1) * SC)
        xt = np.ascontiguousarray(x[rows].T)  # [HID, SC] f32
        b = (c * SC) // S
        in_maps.append(
            dict(
                xtb=xt.astype(bf16),
                xtf=xt,
                wqt=wqt,
                bq=bq,
                wot=wot,
                ktt=kts[b],
                vv=vs[b],
                mb=mbs[b],
            )
        )

    global _LAST_IN_MAPS
    _LAST_IN_MAPS = in_maps
    from concourse import bass2jax

    results = bass2jax.run_bass_via_pjrt(nc, in_maps, n_cores=NC_)

    out = np.empty((B * S, HID), dtype=f32)
    for c in range(NC_):
        out[c * SC : (c + 1) * SC] = results[c]["outt"].T
    return out.reshape(B, S, HID)



# revision 4
# speedup vs baseline: 1.5510x; 1.5510x over previous
"""CrossAttentionLayer kernel for 8x Trainium2 NeuronCores — fp8 DoubleRow.

Problem (hardcoded): B=2, S=4096, HIDDEN=4096, HEADS=32, HEAD_DIM=128,
SLOTS=128, LN eps 1e-5.  out = x + (softmax(LN(x)@Wq.T split-heads @ K.T
/ sqrt(128), masked) @ V merge-heads) @ Wout.T

Strategy: data-parallel over the 8192 (B*S) rows — 1024 rows per core,
core c takes batch c//4.  Transposed dataflow (x.T [k, s] tiles).  The two
4096x4096 projections run in fp8-e4m3 MatmulPerfMode.DoubleRow (0.5
cycles/row = 2x bf16): operands are laid out as [128, 2, N] k-tile pairs,
contraction 256 per matmul.  Weights are host-scaled to max 224 (fp8e4
max-normal 240) and the inverse scale is applied on the psum->sbuf copy
(per-partition scalar from a tiny dram tensor, so the bass program stays
input-independent and cacheable).  x itself is fed to the device as fp8
pairs: LN stats (sums via ones-matmul, also DoubleRow) and the normalized
activations both read it; mean/var from fp8 x are exact to ~0.1%.
Attention proper (scores/softmax/attnV, contraction 128) stays bf16.
Residual is added from a separate f32 x.T stream.  Softmax reciprocal
uses the single-op reciprocal_approx_fast DVE custom op instead of the
~12-pass InstReciprocal.
"""
import numpy as np
import ml_dtypes
import concourse.bass as bass
import concourse.mybir as mybir
import concourse.tile as tile
from concourse.vector_clock import ScopedClock

F32 = mybir.dt.float32
BF16 = mybir.dt.bfloat16
F8 = mybir.dt.float8e4
AF = mybir.ActivationFunctionType
ALU = mybir.AluOpType
DR = mybir.MatmulPerfMode.DoubleRow
E4 = ml_dtypes.float8_e4m3

B, S, HID, HEADS, DH, SLOTS = 2, 4096, 4096, 32, 128, 128
NC_ = 8
SC = B * S // NC_          # rows per core = 1024
KT = HID // 128            # 32 k-tiles
NT = HID // 128            # 32 n-tiles (= heads for Q)
JT = KT // 2               # 16 k-tile pairs (DoubleRow)
NSL = SC // 512            # 2 moving slices of 512
EPS = 1e-5
SCALE = DH ** -0.5

_ws_counter = [0]


def _split_waits(nc, maxw=1):
    """This walrus build rejects >1 sync-wait per instruction: hoist
    extras into same-engine no-ops placed just before the instruction."""
    n = 0
    for f in nc.m.functions:
        for blk in f.blocks:
            insts = list(blk.instructions)
            out, dirty = [], False
            for inst in insts:
                si = inst.sync_info
                waits = list(si.on_wait) if (si is not None and si.on_wait) else []
                if len(waits) > maxw:
                    ups = list(si.on_update or [])
                    for i in range(maxw, len(waits), maxw):
                        _ws_counter[0] += 1
                        nop = mybir.InstNoOp(
                            name=f"I-ws{_ws_counter[0]}", ins=[], outs=[]
                        )
                        nop.engine = inst.engine
                        nop.sync_info = mybir.SyncInfo(
                            on_wait=waits[i : i + maxw], on_update=[]
                        )
                        out.append(nop)
                        n += 1
                    inst.sync_info = mybir.SyncInfo(
                        on_wait=waits[:maxw], on_update=ups
                    )
                    dirty = True
                out.append(inst)
            if dirty:
                blk.instructions = out
    return n


def _patch_tile_drain():
    import concourse.tile as tile_mod

    def _patched(self, tick_clock, wait_clock):
        nc = self.nc
        drain_inst = nc.sync.drain()
        wait_clock.add_sem_waits(
            drain_inst.ins, ScopedClock({None: tick_clock.global_clock})
        )
        inst = drain_inst.ins
        si = inst.sync_info
        waits = list(si.on_wait or []) if si is not None else []
        if len(waits) > 1:
            ups = list(si.on_update or []) if si is not None else []
            inst.sync_info = mybir.SyncInfo(on_wait=waits[:1], on_update=ups)
            for i in range(1, len(waits)):
                nop = nc.sync.nop()
                nop.ins.sync_info = mybir.SyncInfo(
                    on_wait=waits[i : i + 1], on_update=[]
                )
        nc.all_engine_barrier()
        assert self.sems is not None
        popped = nc._tile_sem_poison_stack.pop()
        assert popped is self._sem_poison
        nc.clear_and_free_semaphores(list(self.sems.allocated().values()))
        nc.all_engine_barrier()

    tile_mod.TileContext._drain_and_barrier = _patched


def build_nc():
    _patch_tile_drain()
    nc = bass.Bass()

    xq_in = nc.dram_tensor("xq", [JT, 128, 2, SC], F8, kind="ExternalInput")
    xtf_in = nc.dram_tensor("xtf", [HID, SC], F32, kind="ExternalInput")
    wq_in = nc.dram_tensor("wq4", [NT, 128, KT, 128], F8, kind="ExternalInput")
    bq_in = nc.dram_tensor("bq", [128, NT], F32, kind="ExternalInput")
    wo_in = nc.dram_tensor("wo4", [NT, 128, HEADS, 128], F8, kind="ExternalInput")
    kt_in = nc.dram_tensor("ktt", [DH, HEADS, SLOTS], BF16, kind="ExternalInput")
    v_in = nc.dram_tensor("vv", [SLOTS, HEADS, DH], BF16, kind="ExternalInput")
    mb_in = nc.dram_tensor("mb", [SLOTS, 1], F32, kind="ExternalInput")
    scl_in = nc.dram_tensor("scl", [128, 2], F32, kind="ExternalInput")
    out_t = nc.dram_tensor("outt", [HID, SC], F32, kind="ExternalOutput")

    with tile.TileContext(nc) as tc:
        with tc.tile_pool(name="persist", bufs=1) as P:
            ones8 = P.tile([128, 2, 128], F8, tag="ones8")
            nc.vector.memset(ones8[:], 1.0)
            onesb = P.tile([128, 128], BF16, tag="onesb")
            nc.vector.memset(onesb[:], 1.0)
            eps_t = P.tile([128, 1], F32, tag="eps")
            nc.vector.memset(eps_t[:], EPS)
            kt_all = P.tile([128, HEADS, SLOTS], BF16, tag="kt")
            nc.sync.dma_start(kt_all[:], kt_in[:])
            v_all = P.tile([128, HEADS, DH], BF16, tag="v")
            nc.sync.dma_start(v_all[:], v_in[:])
            mb = P.tile([128, 1], F32, tag="mb")
            nc.sync.dma_start(mb[:], mb_in[:])
            bq = P.tile([128, NT], F32, tag="bq")
            nc.sync.dma_start(bq[:], bq_in[:])
            scl = P.tile([128, 2], F32, tag="scl")
            nc.sync.dma_start(scl[:], scl_in[:])

            xq = []
            for j in range(JT):
                t = P.tile([128, 2, SC], F8, tag=f"xq{j}", name=f"xq{j}")
                nc.sync.dma_start(t[:], xq_in[j])
                xq.append(t)

            rstd_b = P.tile([128, SC], BF16, tag="rstd")
            mrs_b = P.tile([128, SC], BF16, tag="mrs")
            xnp = [
                P.tile([128, 2, SC], F8, tag=f"xn{j}", name=f"xn{j}")
                for j in range(JT)
            ]
            attp = [
                P.tile([128, 2, SC], F8, tag=f"att{j}", name=f"att{j}")
                for j in range(JT)
            ]

            # ---------- phase 1: LN stats (fp8 DoubleRow ones-matmuls) ----
            with (
                tc.tile_pool(name="sqp", bufs=4) as SQ,
                tc.tile_pool(name="stps", bufs=1, space="PSUM") as STP,
                tc.tile_pool(name="stsb", bufs=2) as STS,
            ):
                sum_ps = [
                    STP.tile([128, 512], F32, tag=f"sum{sl}", name=f"sum{sl}")
                    for sl in range(NSL)
                ]
                ssq_ps = [
                    STP.tile([128, 512], F32, tag=f"ssq{sl}", name=f"ssq{sl}")
                    for sl in range(NSL)
                ]
                for j in range(JT):
                    sq8 = SQ.tile([128, 2, SC], F8, tag="sq")
                    nc.scalar.square(sq8[:], xq[j][:])
                    for sl in range(NSL):
                        cs = slice(sl * 512, (sl + 1) * 512)
                        nc.tensor.matmul(
                            sum_ps[sl][:], ones8[:], xq[j][:, :, cs],
                            start=(j == 0), stop=(j == JT - 1), perf_mode=DR,
                        )
                        nc.tensor.matmul(
                            ssq_ps[sl][:], ones8[:], sq8[:, :, cs],
                            start=(j == 0), stop=(j == JT - 1), perf_mode=DR,
                        )
                for sl in range(NSL):
                    cs = slice(sl * 512, (sl + 1) * 512)
                    mean = STS.tile([128, 512], F32, tag="mean")
                    nc.vector.tensor_scalar_mul(mean[:], sum_ps[sl][:], 1.0 / HID)
                    esq = STS.tile([128, 512], F32, tag="esq")
                    nc.vector.tensor_scalar_mul(esq[:], ssq_ps[sl][:], 1.0 / HID)
                    msq = STS.tile([128, 512], F32, tag="msq")
                    nc.vector.tensor_mul(msq[:], mean[:], mean[:])
                    var = STS.tile([128, 512], F32, tag="var")
                    nc.vector.tensor_sub(var[:], esq[:], msq[:])
                    std = STS.tile([128, 512], F32, tag="std")
                    nc.scalar.activation(std[:], var[:], AF.Sqrt, bias=eps_t[:])
                    rr = STS.tile([128, 512], F32, tag="rr")
                    nc.vector.reciprocal(rr[:], std[:])
                    nc.vector.tensor_scalar_mul(rstd_b[:, cs], rr[:], 1.0)
                    nc.vector.tensor_mul(mrs_b[:, cs], mean[:], rr[:])

            # ---------- phase 2: xn (fp8 pair tiles) ----------
            with tc.tile_pool(name="tmpp", bufs=4) as TMP:
                for j in range(JT):
                    for i in range(2):
                        tmp = TMP.tile([128, SC], BF16, tag="tmp")
                        nc.vector.tensor_mul(tmp[:], xq[j][:, i, :], rstd_b[:])
                        nc.vector.tensor_sub(xnp[j][:, i, :], tmp[:], mrs_b[:])

            # ---------- phase 3: per-head Q-proj (DR) + attention ----------
            with (
                tc.tile_pool(name="wq", bufs=2) as WQ,
                tc.tile_pool(name="qps", bufs=2, space="PSUM") as QPS,
                tc.tile_pool(name="qsb", bufs=2) as QSB,
                tc.tile_pool(name="aps", bufs=1, space="PSUM") as APS,
                tc.tile_pool(name="expp", bufs=2) as EXP,
                tc.tile_pool(name="rcp", bufs=2) as RCP,
            ):
                for h in range(HEADS):
                    wq = WQ.tile([128, KT, 128], F8, tag="wq")
                    nc.sync.dma_start(wq[:], wq_in[h])
                    qt_ps = [
                        QPS.tile([128, 512], F32, tag=f"qt{sl}", name=f"qtp{sl}")
                        for sl in range(NSL)
                    ]
                    for j in range(JT):
                        for sl in range(NSL):
                            cs = slice(sl * 512, (sl + 1) * 512)
                            nc.tensor.matmul(
                                qt_ps[sl][:], wq[:, 2 * j : 2 * j + 2, :],
                                xnp[j][:, :, cs],
                                start=(j == 0), stop=(j == JT - 1), perf_mode=DR,
                            )
                    qt = QSB.tile([128, SC], BF16, tag="qt")
                    for sl in range(NSL):
                        cs = slice(sl * 512, (sl + 1) * 512)
                        nc.vector.tensor_scalar(
                            qt[:, cs], qt_ps[sl][:], scl[:, 0:1], bq[:, h : h + 1],
                            ALU.mult, ALU.add,
                        )
                    expt = EXP.tile([128, SC], BF16, tag="expt")
                    for sl in range(NSL):
                        cs = slice(sl * 512, (sl + 1) * 512)
                        sc_ps = APS.tile([128, 512], F32, tag=f"sc{sl}")
                        nc.tensor.matmul(
                            sc_ps[:], kt_all[:, h, :], qt[:, cs],
                            start=True, stop=True,
                        )
                        nc.scalar.activation(
                            expt[:, cs], sc_ps[:], AF.Exp,
                            bias=mb[:], scale=SCALE,
                        )
                    for sl in range(NSL):
                        cs = slice(sl * 512, (sl + 1) * 512)
                        den_ps = APS.tile([128, 512], F32, tag="den")
                        nc.tensor.matmul(
                            den_ps[:], onesb[:], expt[:, cs], start=True, stop=True
                        )
                        rcp = RCP.tile([128, 512], F32, tag="rcp")
                        nc.vector.reciprocal(rcp[:], den_ps[:])
                        at_ps = APS.tile([128, 512], F32, tag="at")
                        nc.tensor.matmul(
                            at_ps[:], v_all[:, h, :], expt[:, cs],
                            start=True, stop=True,
                        )
                        nc.vector.tensor_mul(
                            attp[h // 2][:, h % 2, cs], at_ps[:], rcp[:]
                        )

            # ---------- phase 4: out-proj (DR) + residual ----------
            with (
                tc.tile_pool(name="wo", bufs=2) as WO,
                tc.tile_pool(name="ops", bufs=2, space="PSUM") as OPS,
                tc.tile_pool(name="xrs", bufs=2) as XRS,
                tc.tile_pool(name="osb", bufs=3) as OSB,
            ):
                for nt in range(NT):
                    wo = WO.tile([128, HEADS, 128], F8, tag="wo")
                    nc.sync.dma_start(wo[:], wo_in[nt])
                    xr = XRS.tile([128, SC], F32, tag="xr")
                    nc.sync.dma_start(xr[:], xtf_in[nt * 128 : (nt + 1) * 128, :])
                    o_ps = [
                        OPS.tile([128, 512], F32, tag=f"o{sl}", name=f"op{sl}")
                        for sl in range(NSL)
                    ]
                    for j in range(JT):
                        for sl in range(NSL):
                            cs = slice(sl * 512, (sl + 1) * 512)
                            nc.tensor.matmul(
                                o_ps[sl][:], wo[:, 2 * j : 2 * j + 2, :],
                                attp[j][:, :, cs],
                                start=(j == 0), stop=(j == JT - 1), perf_mode=DR,
                            )
                    for sl in range(NSL):
                        cs = slice(sl * 512, (sl + 1) * 512)
                        osb = OSB.tile([128, 512], F32, tag="osb")
                        nc.vector.scalar_tensor_tensor(
                            osb[:], o_ps[sl][:], scl[:, 1:2], xr[:, cs],
                            ALU.mult, ALU.add,
                        )
                        nc.sync.dma_start(
                            out_t[nt * 128 : (nt + 1) * 128, cs], osb[:]
                        )

    _split_waits(nc)
    return nc


_NC_CACHE = None
_LAST_IN_MAPS = None


def kernel(
    hidden_states, memory_keys, memory_values, attention_mask, Wq, Wout,
    ln_gamma, ln_beta,
):
    global _NC_CACHE
    if _NC_CACHE is None:
        _NC_CACHE = build_nc()
    nc = _NC_CACHE

    f32 = np.float32
    bf16 = ml_dtypes.bfloat16
    x = np.asarray(hidden_states, dtype=f32).reshape(B * S, HID)
    gamma = np.asarray(ln_gamma, dtype=f32)
    beta = np.asarray(ln_beta, dtype=f32)
    Wq = np.asarray(Wq, dtype=f32)
    Wout = np.asarray(Wout, dtype=f32)

    wq_eff = (Wq * gamma[None, :]).T  # [HID(k), HID(n)]
    sq = 224.0 / max(float(np.abs(wq_eff).max()), 1e-30)
    wq4 = np.ascontiguousarray(
        (wq_eff * sq).reshape(KT, 128, NT, 128).transpose(2, 1, 0, 3)
    ).astype(E4)
    bq = np.ascontiguousarray((Wq @ beta).reshape(NT, 128).T).astype(f32)

    wot = Wout.T  # [HID(h-major k), HID(n)]
    so = 224.0 / max(float(np.abs(wot).max()), 1e-30)
    wo4 = np.ascontiguousarray(
        (wot * so).reshape(HEADS, 128, NT, 128).transpose(2, 1, 0, 3)
    ).astype(E4)

    scl = np.empty((128, 2), dtype=f32)
    scl[:, 0] = 1.0 / sq
    scl[:, 1] = 1.0 / so

    kts, vs, mbs = [], [], []
    for b in range(B):
        kb = np.asarray(memory_keys[b], dtype=f32).reshape(SLOTS, HEADS, DH)
        vb = np.asarray(memory_values[b], dtype=f32).reshape(SLOTS, HEADS, DH)
        kts.append(np.ascontiguousarray(kb.transpose(2, 1, 0)).astype(bf16))
        vs.append(np.ascontiguousarray(vb).astype(bf16))
        m = np.asarray(attention_mask[b]).astype(bool)
        mbs.append(np.where(m, 0.0, -1e30).astype(f32).reshape(SLOTS, 1))

    in_maps = []
    for c in range(NC_):
        rows = slice(c * SC, (c + 1) * SC)
        xt = np.ascontiguousarray(x[rows].T)  # [HID, SC] f32
        xq8 = np.ascontiguousarray(
            xt.reshape(JT, 2, 128, SC).transpose(0, 2, 1, 3)
        ).astype(E4)
        b = (c * SC) // S
        in_maps.append(
            dict(
                xq=xq8,
                xtf=xt,
                wq4=wq4,
                bq=bq,
                wo4=wo4,
                ktt=kts[b],
                vv=vs[b],
                mb=mbs[b],
                scl=scl,
            )
        )

    global _LAST_IN_MAPS
    _LAST_IN_MAPS = in_maps
    from concourse import bass2jax

    results = bass2jax.run_bass_via_pjrt(nc, in_maps, n_cores=NC_)

    out = np.empty((B * S, HID), dtype=f32)
    for c in range(NC_):
        out[c * SC : (c + 1) * SC] = results[c]["outt"].T
    return out.reshape(B, S, HID)


# revision 6
# speedup vs baseline: 1.9849x; 1.2797x over previous
"""CrossAttentionLayer kernel for 8x Trainium2 NeuronCores — fp8 DoubleRow v3.

Problem (hardcoded): B=2, S=4096, HIDDEN=4096, HEADS=32, HEAD_DIM=128,
SLOTS=128, LN eps 1e-5.  out = x + (softmax(LN(x)@Wq.T split-heads @ K.T
/ sqrt(128), masked) @ V merge-heads) @ Wout.T

Strategy: data-parallel over the 8192 (B*S) rows — 1024 rows per core.
Transposed dataflow (x.T [k, s] tiles).  Both 4096x4096 projections run in
fp8-e4m3 MatmulPerfMode.DoubleRow (contraction-256 pairs, 2x bf16 rate).

v3 structural points:
  * Q-proj consumes RAW fp8 x (no normalized-activation tensor is ever
    materialized).  LayerNorm is folded into the scores stage:
      scores[t,s] = rstd_s*SX[t,s] - mrs_s*kw1_h[t] + kb_h[t]
    with SX = K_h.T @ (x @ Wq'), kw1_h = K_h @ rowsum(Wq'), kb_h = K_h @
    (Wq@beta) host-precomputed.  The affine is 2 DVE ops per 512-slice;
    the kb/mask term rides the exp bias.  This removes the xn phase that
    serialized the previous version (PE idle 87us) and starts the big
    Q-proj GEMM ~20us into the kernel.
  * All reciprocals run on the Scalar engine as exp(-ln(x)) (and LN rsqrt
    as exp(-0.5*ln(var+eps))) — the DVE InstReciprocal costs 3.3us per
    [128,512] tile and was throttling the per-head pipeline (221us total).
    Exp+Ln live in one ACT table set, so no table switching.
  * Per-head software pipeline: head h-1's denominator/attV matmuls are
    issued between head h's Q-proj and scores matmuls, so ACT/DVE latency
    hides under the in-order PE queue.
  * Weight scales (fp8e4 max-normal 240; weights scaled to max 224) are
    undone via per-partition scalars from a tiny dram tensor, keeping the
    bass program input-independent and cacheable.
"""
import numpy as np
import ml_dtypes
import concourse.bass as bass
import concourse.mybir as mybir
import concourse.tile as tile
from concourse.vector_clock import ScopedClock

F32 = mybir.dt.float32
BF16 = mybir.dt.bfloat16
F8 = mybir.dt.float8e4
AF = mybir.ActivationFunctionType
ALU = mybir.AluOpType
DR = mybir.MatmulPerfMode.DoubleRow
E4 = ml_dtypes.float8_e4m3

B, S, HID, HEADS, DH, SLOTS = 2, 4096, 4096, 32, 128, 128
NC_ = 8
SC = B * S // NC_          # rows per core = 1024
KT = HID // 128            # 32 k-tiles
NT = HID // 128            # 32 n-tiles (= heads for Q)
JT = KT // 2               # 16 k-tile pairs (DoubleRow)
NSL = SC // 512            # 2 moving slices of 512
EPS = 1e-5
SCALE = DH ** -0.5

_ws_counter = [0]


def _split_waits(nc, maxw=1):
    """This walrus build rejects >1 sync-wait per instruction: hoist
    extras into same-engine no-ops placed just before the instruction."""
    n = 0
    for f in nc.m.functions:
        for blk in f.blocks:
            insts = list(blk.instructions)
            out, dirty = [], False
            for inst in insts:
                si = inst.sync_info
                waits = list(si.on_wait) if (si is not None and si.on_wait) else []
                if len(waits) > maxw:
                    ups = list(si.on_update or [])
                    for i in range(maxw, len(waits), maxw):
                        _ws_counter[0] += 1
                        nop = mybir.InstNoOp(
                            name=f"I-ws{_ws_counter[0]}", ins=[], outs=[]
                        )
                        nop.engine = inst.engine
                        nop.sync_info = mybir.SyncInfo(
                            on_wait=waits[i : i + maxw], on_update=[]
                        )
                        out.append(nop)
                        n += 1
                    inst.sync_info = mybir.SyncInfo(
                        on_wait=waits[:maxw], on_update=ups
                    )
                    dirty = True
                out.append(inst)
            if dirty:
                blk.instructions = out
    return n


def _patch_tile_drain():
    import concourse.tile as tile_mod

    def _patched(self, tick_clock, wait_clock):
        nc = self.nc
        drain_inst = nc.sync.drain()
        wait_clock.add_sem_waits(
            drain_inst.ins, ScopedClock({None: tick_clock.global_clock})
        )
        inst = drain_inst.ins
        si = inst.sync_info
        waits = list(si.on_wait or []) if si is not None else []
        if len(waits) > 1:
            ups = list(si.on_update or []) if si is not None else []
            inst.sync_info = mybir.SyncInfo(on_wait=waits[:1], on_update=ups)
            for i in range(1, len(waits)):
                nop = nc.sync.nop()
                nop.ins.sync_info = mybir.SyncInfo(
                    on_wait=waits[i : i + 1], on_update=[]
                )
        nc.all_engine_barrier()
        assert self.sems is not None
        popped = nc._tile_sem_poison_stack.pop()
        assert popped is self._sem_poison
        nc.clear_and_free_semaphores(list(self.sems.allocated().values()))
        nc.all_engine_barrier()

    tile_mod.TileContext._drain_and_barrier = _patched


def build_nc():
    _patch_tile_drain()
    nc = bass.Bass()

    xq_in = nc.dram_tensor("xq", [JT, 128, 2, SC], F8, kind="ExternalInput")
    xtf_in = nc.dram_tensor("xtf", [HID, SC], F32, kind="ExternalInput")
    wq_in = nc.dram_tensor("wq4", [NT, 128, KT, 128], F8, kind="ExternalInput")
    wo_in = nc.dram_tensor("wo4", [NT, 128, HEADS, 128], F8, kind="ExternalInput")
    kt_in = nc.dram_tensor("ktt", [DH, HEADS, SLOTS], BF16, kind="ExternalInput")
    v_in = nc.dram_tensor("vv", [SLOTS, HEADS, DH], BF16, kind="ExternalInput")
    eb_in = nc.dram_tensor("eb", [SLOTS, HEADS], F32, kind="ExternalInput")
    kw1_in = nc.dram_tensor("kw1", [SLOTS, HEADS], F32, kind="ExternalInput")
    scl_in = nc.dram_tensor("scl", [128, 2], F32, kind="ExternalInput")
    out_t = nc.dram_tensor("outt", [HID, SC], F32, kind="ExternalOutput")

    with tile.TileContext(nc) as tc:
        with tc.tile_pool(name="persist", bufs=1) as P:
            ones8 = P.tile([128, 2, 128], F8, tag="ones8")
            nc.vector.memset(ones8[:], 1.0)
            onesb = P.tile([128, 128], BF16, tag="onesb")
            nc.vector.memset(onesb[:], 1.0)
            eps_t = P.tile([128, 1], F32, tag="eps")
            nc.vector.memset(eps_t[:], EPS)
            kt_all = P.tile([128, HEADS, SLOTS], BF16, tag="kt")
            nc.sync.dma_start(kt_all[:], kt_in[:])
            v_all = P.tile([128, HEADS, DH], BF16, tag="v")
            nc.sync.dma_start(v_all[:], v_in[:])
            eb_t = P.tile([128, HEADS], F32, tag="eb")
            nc.sync.dma_start(eb_t[:], eb_in[:])
            kw1_t = P.tile([128, HEADS], F32, tag="kw1")
            nc.sync.dma_start(kw1_t[:], kw1_in[:])
            scl = P.tile([128, 2], F32, tag="scl")
            nc.sync.dma_start(scl[:], scl_in[:])

            xq = []
            for j in range(JT):
                t = P.tile([128, 2, SC], F8, tag=f"xq{j}", name=f"xq{j}")
                nc.sync.dma_start(t[:], xq_in[j])
                xq.append(t)

            rstd_b = P.tile([128, SC], BF16, tag="rstd")
            mrs_b = P.tile([128, SC], BF16, tag="mrs")
            attp = [
                P.tile([128, 2, SC], F8, tag=f"att{j}", name=f"att{j}")
                for j in range(JT)
            ]

            with (
                # long-lived streaming pools first: their SBUF must not
                # alias the short stats pools (aliasing = serialization)
                tc.tile_pool(name="wq", bufs=2) as WQ,
                tc.tile_pool(name="qsb", bufs=2) as QSB,
                tc.tile_pool(name="t1p", bufs=2) as T1P,
                tc.tile_pool(name="scbp", bufs=2) as SCBP,
                tc.tile_pool(name="expp", bufs=2) as EXP,
                tc.tile_pool(name="ltp", bufs=2) as LTP,
                tc.tile_pool(name="rcp", bufs=2) as RCP,
                tc.tile_pool(name="wo", bufs=2) as WO,
                tc.tile_pool(name="xrs", bufs=2) as XRS,
                tc.tile_pool(name="osb", bufs=3) as OSB,
                tc.tile_pool(name="qps", bufs=1, space="PSUM") as QPS,
            ):
                # ---------- LN stats (fp8 DoubleRow ones-matmuls) ----------
                with (
                    tc.tile_pool(name="sqp", bufs=4) as SQ,
                    tc.tile_pool(name="stps", bufs=1, space="PSUM") as STP,
                    tc.tile_pool(name="stsb", bufs=2) as STS,
                ):
                    sum_ps = [
                        STP.tile([128, 512], F32, tag=f"sum{sl}", name=f"sum{sl}")
                        for sl in range(NSL)
                    ]
                    ssq_ps = [
                        STP.tile([128, 512], F32, tag=f"ssq{sl}", name=f"ssq{sl}")
                        for sl in range(NSL)
                    ]
                    for j in range(JT):
                        sq8 = SQ.tile([128, 2, SC], F8, tag="sq")
                        nc.scalar.square(sq8[:], xq[j][:])
                        for sl in range(NSL):
                            cs = slice(sl * 512, (sl + 1) * 512)
                            nc.tensor.matmul(
                                sum_ps[sl][:], ones8[:], xq[j][:, :, cs],
                                start=(j == 0), stop=(j == JT - 1), perf_mode=DR,
                            )
                            nc.tensor.matmul(
                                ssq_ps[sl][:], ones8[:], sq8[:, :, cs],
                                start=(j == 0), stop=(j == JT - 1), perf_mode=DR,
                            )
                    for sl in range(NSL):
                        cs = slice(sl * 512, (sl + 1) * 512)
                        mean = STS.tile([128, 512], F32, tag="mean")
                        nc.vector.tensor_scalar_mul(mean[:], sum_ps[sl][:], 1.0 / HID)
                        esq = STS.tile([128, 512], F32, tag="esq")
                        nc.vector.tensor_scalar_mul(esq[:], ssq_ps[sl][:], 1.0 / HID)
                        msq = STS.tile([128, 512], F32, tag="msq")
                        nc.vector.tensor_mul(msq[:], mean[:], mean[:])
                        var = STS.tile([128, 512], F32, tag="var")
                        nc.vector.tensor_sub(var[:], esq[:], msq[:])
                        lv = STS.tile([128, 512], F32, tag="lv")
                        nc.scalar.activation(lv[:], var[:], AF.Ln, bias=eps_t[:])
                        nc.scalar.activation(rstd_b[:, cs], lv[:], AF.Exp, scale=-0.5)
                        nc.vector.tensor_mul(mrs_b[:, cs], mean[:], rstd_b[:, cs])

                # ---------- per-head Q-proj (DR) + attention, pipelined ----
                with tc.tile_pool(name="aps", bufs=1, space="PSUM") as APS:

                    def attn_tail(h, expt):
                        for sl in range(NSL):
                            cs = slice(sl * 512, (sl + 1) * 512)
                            den_ps = APS.tile([128, 512], F32, tag=f"den{sl}")
                            nc.tensor.matmul(
                                den_ps[:], onesb[:], expt[:, cs],
                                start=True, stop=True,
                            )
                            at_ps = APS.tile([128, 512], F32, tag=f"at{sl}")
                            nc.tensor.matmul(
                                at_ps[:], v_all[:, h, :], expt[:, cs],
                                start=True, stop=True,
                            )
                            lt = LTP.tile([128, 512], F32, tag="lt")
                            nc.scalar.activation(lt[:], den_ps[:], AF.Ln)
                            rcp = RCP.tile([128, 512], F32, tag="rcp")
                            nc.scalar.activation(rcp[:], lt[:], AF.Exp, scale=-1.0)
                            nc.vector.tensor_mul(
                                attp[h // 2][:, h % 2, cs], at_ps[:], rcp[:]
                            )

                    prev = None
                    for h in range(HEADS):
                        wq = WQ.tile([128, KT, 128], F8, tag="wq")
                        nc.sync.dma_start(wq[:], wq_in[h])
                        qt_ps = [
                            QPS.tile([128, 512], F32, tag=f"qt{sl}", name=f"qtp{sl}")
                            for sl in range(NSL)
                        ]
                        for j in range(JT):
                            for sl in range(NSL):
                                cs = slice(sl * 512, (sl + 1) * 512)
                                nc.tensor.matmul(
                                    qt_ps[sl][:], wq[:, 2 * j : 2 * j + 2, :],
                                    xq[j][:, :, cs],
                                    start=(j == 0), stop=(j == JT - 1),
                                    perf_mode=DR,
                                )
                        qt = QSB.tile([128, SC], BF16, tag="qt")
                        for sl in range(NSL):
                            cs = slice(sl * 512, (sl + 1) * 512)
                            nc.vector.tensor_scalar_mul(
                                qt[:, cs], qt_ps[sl][:], scl[:, 0:1]
                            )
                        # head h-1 tail goes to the PE between our Q-proj and
                        # scores matmuls: its exp/den inputs are long ready, so
                        # the in-order PE queue never waits on ACT/DVE here.
                        if prev is not None:
                            attn_tail(*prev)
                        sc_ps = [
                            APS.tile([128, 512], F32, tag=f"sc{sl}", name=f"scp{sl}")
                            for sl in range(NSL)
                        ]
                        for sl in range(NSL):
                            cs = slice(sl * 512, (sl + 1) * 512)
                            nc.tensor.matmul(
                                sc_ps[sl][:], kt_all[:, h, :], qt[:, cs],
                                start=True, stop=True,
                            )
                        scb = SCBP.tile([128, SC], BF16, tag="scb")
                        expt = EXP.tile([128, SC], BF16, tag="expt")
                        for sl in range(NSL):
                            cs = slice(sl * 512, (sl + 1) * 512)
                            t1 = T1P.tile([128, 512], BF16, tag="t1")
                            nc.vector.tensor_mul(t1[:], sc_ps[sl][:], rstd_b[:, cs])
                            nc.vector.scalar_tensor_tensor(
                                scb[:, cs], mrs_b[:, cs], kw1_t[:, h : h + 1],
                                t1[:], ALU.mult, ALU.subtract,
                            )
                            nc.scalar.activation(
                                expt[:, cs], scb[:, cs], AF.Exp,
                                bias=eb_t[:, h : h + 1], scale=-SCALE,
                            )
                        prev = (h, expt)
                    attn_tail(*prev)

                # ---------- out-proj (DR) + residual ----------
                with tc.tile_pool(name="ops", bufs=2, space="PSUM") as OPS:
                    for nt in range(NT):
                        wo = WO.tile([128, HEADS, 128], F8, tag="wo")
                        nc.sync.dma_start(wo[:], wo_in[nt])
                        xr = XRS.tile([128, SC], F32, tag="xr")
                        nc.sync.dma_start(
                            xr[:], xtf_in[nt * 128 : (nt + 1) * 128, :]
                        )
                        o_ps = [
                            OPS.tile([128, 512], F32, tag=f"o{sl}", name=f"op{sl}")
                            for sl in range(NSL)
                        ]
                        for j in range(JT):
                            for sl in range(NSL):
                                cs = slice(sl * 512, (sl + 1) * 512)
                                nc.tensor.matmul(
                                    o_ps[sl][:], wo[:, 2 * j : 2 * j + 2, :],
                                    attp[j][:, :, cs],
                                    start=(j == 0), stop=(j == JT - 1),
                                    perf_mode=DR,
                                )
                        for sl in range(NSL):
                            cs = slice(sl * 512, (sl + 1) * 512)
                            osb = OSB.tile([128, 512], F32, tag="osb")
                            nc.vector.scalar_tensor_tensor(
                                osb[:], o_ps[sl][:], scl[:, 1:2], xr[:, cs],
                                ALU.mult, ALU.add,
                            )
                            nc.sync.dma_start(
                                out_t[nt * 128 : (nt + 1) * 128, cs], osb[:]
                            )

    _split_waits(nc)
    return nc


_NC_CACHE = None
_LAST_IN_MAPS = None


def kernel(
    hidden_states, memory_keys, memory_values, attention_mask, Wq, Wout,
    ln_gamma, ln_beta,
):
    global _NC_CACHE
    if _NC_CACHE is None:
        _NC_CACHE = build_nc()
    nc = _NC_CACHE

    f32 = np.float32
    bf16 = ml_dtypes.bfloat16
    x = np.asarray(hidden_states, dtype=f32).reshape(B * S, HID)
    gamma = np.asarray(ln_gamma, dtype=f32)
    beta = np.asarray(ln_beta, dtype=f32)
    Wq = np.asarray(Wq, dtype=f32)
    Wout = np.asarray(Wout, dtype=f32)

    wq_nk = Wq * gamma[None, :]      # [n, k]
    sq = 224.0 / max(float(np.abs(wq_nk).max()), 1e-30)
    wq4 = np.ascontiguousarray(
        (wq_nk.T * sq).reshape(KT, 128, NT, 128).transpose(2, 1, 0, 3)
    ).astype(E4)
    bq = Wq @ beta                   # [n]
    w1 = wq_nk.sum(axis=1)           # [n] = sum_k Wq'[n,k]

    wot = Wout.T                     # [HID(h-major k), HID(n)]
    so = 224.0 / max(float(np.abs(wot).max()), 1e-30)
    wo4 = np.ascontiguousarray(
        (wot * so).reshape(HEADS, 128, NT, 128).transpose(2, 1, 0, 3)
    ).astype(E4)

    scl = np.empty((128, 2), dtype=f32)
    scl[:, 0] = 1.0 / sq
    scl[:, 1] = 1.0 / so

    w1h = w1.reshape(HEADS, DH)
    bqh = bq.reshape(HEADS, DH)
    kts, vs, ebs, kw1s = [], [], [], []
    for b in range(B):
        kb = np.asarray(memory_keys[b], dtype=f32).reshape(SLOTS, HEADS, DH)
        vb = np.asarray(memory_values[b], dtype=f32).reshape(SLOTS, HEADS, DH)
        kts.append(np.ascontiguousarray(kb.transpose(2, 1, 0)).astype(bf16))
        vs.append(np.ascontiguousarray(vb).astype(bf16))
        m = np.asarray(attention_mask[b]).astype(bool)
        mb = np.where(m, 0.0, -1e30).astype(f32)          # [SLOTS]
        kw1s.append(
            np.ascontiguousarray(np.einsum("thd,hd->th", kb, w1h)).astype(f32)
        )
        ebs.append(
            np.ascontiguousarray(
                mb[:, None] + SCALE * np.einsum("thd,hd->th", kb, bqh)
            ).astype(f32)
        )

    in_maps = []
    for c in range(NC_):
        rows = slice(c * SC, (c + 1) * SC)
        xt = np.ascontiguousarray(x[rows].T)  # [HID, SC] f32
        xq8 = np.ascontiguousarray(
            xt.reshape(JT, 2, 128, SC).transpose(0, 2, 1, 3)
        ).astype(E4)
        b = (c * SC) // S
        in_maps.append(
            dict(
                xq=xq8,
                xtf=xt,
                wq4=wq4,
                wo4=wo4,
                ktt=kts[b],
                vv=vs[b],
                eb=ebs[b],
                kw1=kw1s[b],
                scl=scl,
            )
        )

    global _LAST_IN_MAPS
    _LAST_IN_MAPS = in_maps
    from concourse import bass2jax

    results = bass2jax.run_bass_via_pjrt(nc, in_maps, n_cores=NC_)

    out = np.empty((B * S, HID), dtype=f32)
    for c in range(NC_):
        out[c * SC : (c + 1) * SC] = results[c]["outt"].T
    return out.reshape(B, S, HID)
